# revision 6
# baseline (speedup 1.0000x reference)
"""Brute-force L2 1-NN on 8 TRN2 NeuronCores.

Problem: x [4096, 256], prototypes [32768, 256] -> prototypes[argmin_j ||x-p_j||^2]

Strategy (prototype-sharded SPMD, no collectives):
  - Each core owns a 4096-row shard of the prototype bank; queries replicated.
  - Scores via TensorE fp32r matmuls in [q_part, j_free] orientation with an
    augmented contraction that folds in the |p|^2 term:
      c'[q, j] = x.p - 0.5|p|^2   (argmax_j c' == argmin_j ||x-p||^2)
    K = 256 (two 128-chunks) + one aug chunk (ones row in x^T, -0.5|p|^2 row
    in P^T). fp32r runs at ~1 cycle/column (measured 227 ns per 512-wide
    matmul warm) with ~1.5e-2 absolute error on these magnitudes.
  - VectorE tensor_reduce(max) straight from PSUM at 16-wide granularity:
    one op per 4-bank half -> per-16-chunk maxes m[q, 256] per core.
    No positions are extracted on device.
  - Host: global argmax over 8x256 chunk maxes per query, exact float64
    rescore of the winning 16-wide chunk (+ any chunk within DELTA), then
    row gather. ~20 MFLOP on host.
"""

import sys
import types

sys.path.insert(0, "/opt/trn_rl_repo")


def _install_ntff_hook():
    try:
        from trn_agent_boot.trn_boot import _ntff_profile_via_ctypes
    except ImportError:
        return
    try:
        hook = _ntff_profile_via_ctypes("/opt/axon/libaxon_pjrt.so")
    except OSError:
        return
    mod = types.ModuleType("antenv.axon_hooks")
    _h = [hook]
    mod.get_axon_ntff_profile_hook = lambda: _h[0]
    mod.set_axon_ntff_profile_hook = lambda h: _h.__setitem__(0, h)
    sys.modules["antenv.axon_hooks"] = mod
    import antenv

    antenv.axon_hooks = mod


_install_ntff_hook()

import numpy as np
import concourse.bass as bass
import concourse.mybir as mybir
import concourse.tile as tile
from concourse import bacc
from concourse.bass_utils import run_bass_kernel_spmd

B, N, D = 4096, 32768, 256
NCORES = 8
NLOC = N // NCORES  # 4096 prototypes per core
QT = 128  # queries per tile
NQT = B // QT  # 32 query tiles
JC = 512  # j-chunk width (one psum bank)
NJC = NLOC // JC  # 8 banks-worth per core
G = 16  # reduce granularity (chunk width for host rescore)
NG = NLOC // G  # 256 chunk maxes per core


def build(nqt=NQT, njc=NJC):
    """Build the per-core Bass graph. nqt/njc shrinkable for simulation."""
    f32 = mybir.dt.float32
    f32r = mybir.dt.float32r
    nloc = njc * JC
    b = nqt * QT
    hf = max(1, njc // 2)  # banks per psum half
    ng = nloc // G

    nc = bacc.Bacc("TRN2", target_bir_lowering=False, debug=False, num_devices=NCORES)
    xT_d = nc.dram_tensor("xT", [2, 128, b], f32r, kind="ExternalInput").ap()
    pT_d = nc.dram_tensor("pT", [2, 128, nloc], f32r, kind="ExternalInput").ap()
    xa_d = nc.dram_tensor("xa", [8, b], f32r, kind="ExternalInput").ap()
    pa_d = nc.dram_tensor("pa", [8, nloc], f32r, kind="ExternalInput").ap()
    m_out = nc.dram_tensor("m", [nqt, QT, ng], f32, kind="ExternalOutput").ap()

    with tile.TileContext(nc) as tc:
        with (
            tc.tile_pool(name="persist", bufs=1) as pp,
            tc.tile_pool(name="small", bufs=4) as sp,
            tc.tile_pool(name="ps", bufs=2, space="PSUM") as ps,
        ):
            xT_sb = pp.tile([128, 2, b], f32r)
            pT_sb = pp.tile([128, 2, nloc], f32r)
            xa_sb = pp.tile([8, b], f32r)
            pa_sb = pp.tile([8, nloc], f32r)
            for k in range(2):
                nc.sync.dma_start(xT_sb[:, k, :], xT_d[k])
                nc.sync.dma_start(pT_sb[:, k, :], pT_d[k])
            nc.sync.dma_start(xa_sb[:], xa_d)
            nc.sync.dma_start(pa_sb[:], pa_d)

            gph = hf * JC // G  # chunk maxes per half
            for qt in range(nqt):
                qs = bass.ts(qt, QT)
                m_sb = sp.tile([QT, ng], f32, tag="m")
                for h in range(njc // hf):
                    psum_h = ps.tile([QT, hf, JC], f32, tag="psb", name=f"ps{qt}_{h}")
                    for jc in range(hf):
                        for k in range(3):
                            lhs = xa_sb[:, qs] if k == 2 else xT_sb[:, k, qs]
                            rhs = (
                                pa_sb[:, bass.ts(h * hf + jc, JC)]
                                if k == 2
                                else pT_sb[:, k, bass.ts(h * hf + jc, JC)]
                            )
                            nc.tensor.matmul(
                                psum_h[:, jc, :],
                                lhs,
                                rhs,
                                start=(k == 0),
                                stop=(k == 2),
                            )
                    # per-16-wide-chunk maxes, straight from PSUM
                    nc.vector.tensor_reduce(
                        m_sb[:, h * gph : (h + 1) * gph],
                        psum_h[:].rearrange("q c j -> q (c j)").rearrange(
                            "q (g i) -> q g i", i=G
                        ),
                        axis=mybir.AxisListType.X,
                        op=mybir.AluOpType.max,
                    )
                nc.sync.dma_start(m_out[qt], m_sb[:])
    nc.compile()
    return nc


def _prep_inputs(x, prototypes):
    """Host-side shard prep: transposes, aug rows, sharding."""
    xT = np.ascontiguousarray(x.T).reshape(2, 128, B)
    xa = np.zeros((8, B), dtype=np.float32)
    xa[0] = 1.0
    in_maps = []
    for c in range(NCORES):
        P = prototypes[c * NLOC : (c + 1) * NLOC]
        pT = np.ascontiguousarray(P.T).reshape(2, 128, NLOC)
        pa = np.zeros((8, NLOC), dtype=np.float32)
        pa[0] = -0.5 * np.einsum("jd,jd->j", P, P)
        in_maps.append({"xT": xT, "pT": pT, "xa": xa, "pa": pa})
    return in_maps


_NC_CACHE = {}

# Candidate threshold: chunks whose measured max is within DELTA of the
# best measured max get exact-rescored. fp32r error here is <~2e-2 abs.
DELTA = 0.15


def kernel(x: np.ndarray, prototypes: np.ndarray) -> np.ndarray:
    x = np.asarray(x, dtype=np.float32)
    prototypes = np.asarray(prototypes, dtype=np.float32)
    assert x.shape == (B, D) and prototypes.shape == (N, D)

    if "nc" not in _NC_CACHE:
        _NC_CACHE["nc"] = build()
    nc = _NC_CACHE["nc"]
    in_maps = _prep_inputs(x, prototypes)
    res = run_bass_kernel_spmd(nc, in_maps, core_ids=list(range(NCORES)))
    _NC_CACHE["last_results"] = res

    # m[c, q, g]: max of c' over 16-wide chunk g of core c (fp32r-accurate)
    m_all = np.stack([res.results[c]["m"].reshape(B, NG) for c in range(NCORES)])
    m_flat = np.transpose(m_all, (1, 0, 2)).reshape(B, NCORES * NG)  # [B, 2048]

    best = m_flat.max(axis=1, keepdims=True)
    qs, gs = np.nonzero(m_flat >= best - DELTA)  # candidate (query, chunk) pairs

    # exact rescore of candidate chunks in float64
    cand_j = (gs[:, None] * G + np.arange(G)[None, :]).reshape(-1)  # [nc*G]
    qq = np.repeat(qs, G)
    pc = prototypes[cand_j].astype(np.float64)
    xc = x[qq].astype(np.float64)
    c_exact = np.einsum("ij,ij->i", pc, xc) - 0.5 * np.einsum("ij,ij->i", pc, pc)
    order = np.lexsort((cand_j, -c_exact, qq))
    qs_o = qq[order]
    first = np.unique(qs_o, return_index=True)[1]
    out_idx = np.empty(B, dtype=np.int64)
    out_idx[qs_o[first]] = cand_j[order][first]

    return prototypes[out_idx]


if __name__ == "__main__":
    rng = np.random.default_rng(0)
    x = rng.standard_normal((B, D), dtype=np.float32)
    p = rng.standard_normal((N, D), dtype=np.float32)
    out = kernel(x, p)
    print("out", out.shape, out.dtype)


# revision 7
# speedup vs baseline: 2.3593x; 2.3593x over previous
"""Brute-force L2 1-NN on 8 TRN2 NeuronCores.

Problem: x [4096, 256], prototypes [32768, 256] -> prototypes[argmin_j ||x-p_j||^2]

Strategy (prototype-sharded SPMD, no collectives):
  - Each core owns a 4096-row shard of the prototype bank; queries replicated.
  - Scores via TensorE fp32r matmuls in [q_part, j_free] orientation with an
    augmented contraction that folds in the |p|^2 term:
      c'[q, j] = x.p - 0.5|p|^2   (argmax_j c' == argmin_j ||x-p||^2)
    K = 256 (two 128-chunks) + one aug chunk (ones row in x^T, -0.5|p|^2 row
    in P^T). fp32r runs at ~1 cycle/column (measured 227 ns per 512-wide
    matmul warm) with ~1.5e-2 absolute error on these magnitudes.
  - VectorE tensor_reduce(max) straight from PSUM at 16-wide granularity:
    one op per 4-bank half -> per-16-chunk maxes m[q, 256] per core.
    No positions are extracted on device.
  - Host: global argmax over 8x256 chunk maxes per query, exact float64
    rescore of the winning 16-wide chunk (+ any chunk within DELTA), then
    row gather. ~20 MFLOP on host.
"""

import sys
import types

sys.path.insert(0, "/opt/trn_rl_repo")


def _install_ntff_hook():
    try:
        from trn_agent_boot.trn_boot import _ntff_profile_via_ctypes
    except ImportError:
        return
    try:
        hook = _ntff_profile_via_ctypes("/opt/axon/libaxon_pjrt.so")
    except OSError:
        return
    mod = types.ModuleType("antenv.axon_hooks")
    _h = [hook]
    mod.get_axon_ntff_profile_hook = lambda: _h[0]
    mod.set_axon_ntff_profile_hook = lambda h: _h.__setitem__(0, h)
    sys.modules["antenv.axon_hooks"] = mod
    import antenv

    antenv.axon_hooks = mod


_install_ntff_hook()

import numpy as np
import concourse.bass as bass
import concourse.mybir as mybir
import concourse.tile as tile
from concourse import bacc
from concourse.bass_utils import run_bass_kernel_spmd

B, N, D = 4096, 32768, 256
NCORES = 8
NLOC = N // NCORES  # 4096 prototypes per core
QT = 128  # queries per tile
NQT = B // QT  # 32 query tiles
JC = 512  # j-chunk width (one psum bank)
NJC = NLOC // JC  # 8 banks-worth per core
G = 16  # reduce granularity (chunk width for host rescore)
NG = NLOC // G  # 256 chunk maxes per core


def build(nqt=NQT, njc=NJC):
    """Build the per-core Bass graph. nqt/njc shrinkable for simulation."""
    f32 = mybir.dt.float32
    f32r = mybir.dt.float32r
    nloc = njc * JC
    b = nqt * QT
    hf = max(1, njc // 2)  # banks per psum half
    ng = nloc // G

    nc = bacc.Bacc("TRN2", target_bir_lowering=False, debug=False, num_devices=NCORES)
    xT_d = nc.dram_tensor("xT", [2, 128, b], f32r, kind="ExternalInput").ap()
    pT_d = nc.dram_tensor("pT", [2, 128, nloc], f32r, kind="ExternalInput").ap()
    xa_d = nc.dram_tensor("xa", [128, b], f32r, kind="ExternalInput").ap()
    pa_d = nc.dram_tensor("pa", [128, nloc], f32r, kind="ExternalInput").ap()
    m_out = nc.dram_tensor("m", [nqt, QT, ng], f32, kind="ExternalOutput").ap()

    with tile.TileContext(nc) as tc:
        with (
            tc.tile_pool(name="persist", bufs=1) as pp,
            tc.tile_pool(name="small", bufs=4) as sp,
            tc.tile_pool(name="ps", bufs=2, space="PSUM") as ps,
        ):
            xT_sb = pp.tile([128, 2, b], f32r)
            pT_sb = pp.tile([128, 2, nloc], f32r)
            xa_sb = pp.tile([128, b], f32r)
            pa_sb = pp.tile([128, nloc], f32r)
            for k in range(2):
                nc.sync.dma_start(xT_sb[:, k, :], xT_d[k])
                nc.sync.dma_start(pT_sb[:, k, :], pT_d[k])
            nc.sync.dma_start(xa_sb[:], xa_d)
            nc.sync.dma_start(pa_sb[:], pa_d)

            gph = hf * JC // G  # chunk maxes per half
            for qt in range(nqt):
                qs = bass.ts(qt, QT)
                m_sb = sp.tile([QT, ng], f32, tag="m")
                for h in range(njc // hf):
                    psum_h = ps.tile([QT, hf, JC], f32, tag="psb", name=f"ps{qt}_{h}")
                    for jc in range(hf):
                        for k in range(3):
                            lhs = xa_sb[:, qs] if k == 2 else xT_sb[:, k, qs]
                            rhs = (
                                pa_sb[:, bass.ts(h * hf + jc, JC)]
                                if k == 2
                                else pT_sb[:, k, bass.ts(h * hf + jc, JC)]
                            )
                            nc.tensor.matmul(
                                psum_h[:, jc, :],
                                lhs,
                                rhs,
                                start=(k == 0),
                                stop=(k == 2),
                            )
                    # per-16-wide-chunk maxes, straight from PSUM
                    nc.vector.tensor_reduce(
                        m_sb[:, h * gph : (h + 1) * gph],
                        psum_h[:].rearrange("q c j -> q (c j)").rearrange(
                            "q (g i) -> q g i", i=G
                        ),
                        axis=mybir.AxisListType.X,
                        op=mybir.AluOpType.max,
                    )
                nc.sync.dma_start(m_out[qt], m_sb[:])
    nc.compile()
    return nc


def _prep_inputs(x, prototypes):
    """Host-side shard prep: transposes, aug rows, sharding."""
    xT = np.ascontiguousarray(x.T).reshape(2, 128, B)
    xa = np.zeros((128, B), dtype=np.float32)
    xa[0] = 1.0
    in_maps = []
    for c in range(NCORES):
        P = prototypes[c * NLOC : (c + 1) * NLOC]
        pT = np.ascontiguousarray(P.T).reshape(2, 128, NLOC)
        pa = np.zeros((128, NLOC), dtype=np.float32)
        pa[0] = -0.5 * np.einsum("jd,jd->j", P, P)
        in_maps.append({"xT": xT, "pT": pT, "xa": xa, "pa": pa})
    return in_maps


_NC_CACHE = {}

# Candidate threshold: chunks whose measured max is within DELTA of the
# best measured max get exact-rescored. fp32r error here is <~2e-2 abs.
DELTA = 0.15


def kernel(x: np.ndarray, prototypes: np.ndarray) -> np.ndarray:
    x = np.asarray(x, dtype=np.float32)
    prototypes = np.asarray(prototypes, dtype=np.float32)
    assert x.shape == (B, D) and prototypes.shape == (N, D)

    if "nc" not in _NC_CACHE:
        _NC_CACHE["nc"] = build()
    nc = _NC_CACHE["nc"]
    in_maps = _prep_inputs(x, prototypes)
    res = run_bass_kernel_spmd(nc, in_maps, core_ids=list(range(NCORES)))
    _NC_CACHE["last_results"] = res

    # m[c, q, g]: max of c' over 16-wide chunk g of core c (fp32r-accurate)
    m_all = np.stack([res.results[c]["m"].reshape(B, NG) for c in range(NCORES)])
    m_flat = np.transpose(m_all, (1, 0, 2)).reshape(B, NCORES * NG)  # [B, 2048]

    best = m_flat.max(axis=1, keepdims=True)
    qs, gs = np.nonzero(m_flat >= best - DELTA)  # candidate (query, chunk) pairs

    # exact rescore of candidate chunks in float64
    cand_j = (gs[:, None] * G + np.arange(G)[None, :]).reshape(-1)  # [nc*G]
    qq = np.repeat(qs, G)
    pc = prototypes[cand_j].astype(np.float64)
    xc = x[qq].astype(np.float64)
    c_exact = np.einsum("ij,ij->i", pc, xc) - 0.5 * np.einsum("ij,ij->i", pc, pc)
    order = np.lexsort((cand_j, -c_exact, qq))
    qs_o = qq[order]
    first = np.unique(qs_o, return_index=True)[1]
    out_idx = np.empty(B, dtype=np.int64)
    out_idx[qs_o[first]] = cand_j[order][first]

    return prototypes[out_idx]


if __name__ == "__main__":
    rng = np.random.default_rng(0)
    x = rng.standard_normal((B, D), dtype=np.float32)
    p = rng.standard_normal((N, D), dtype=np.float32)
    out = kernel(x, p)
    print("out", out.shape, out.dtype)


# revision 8
# speedup vs baseline: 3.4049x; 1.4431x over previous
"""Brute-force L2 1-NN on 8 TRN2 NeuronCores.

Problem: x [4096, 256], prototypes [32768, 256] -> prototypes[argmin_j ||x-p_j||^2]

Strategy (prototype-sharded SPMD, no collectives):
  - Host sorts the prototype bank by |p|^2 and shards the sorted order across
    8 cores (each core gets a contiguous |p|^2 band); queries replicated.
  - Device computes raw scores s[q, j] = x.p via TensorE fp32r matmuls
    ([q_part, j_free], K=256 as two 128-chunks; fp32r measured at 227 ns per
    128x128x512 matmul warm, abs err <~2e-2 on these magnitudes).
  - VectorE tensor_reduce(max) straight from PSUM at G=16 granularity:
    m[q, g] = max of x.p over each 16-wide sorted chunk. No positions and no
    |p|^2 correction on device.
  - Host: for chunk g, the true max of c' = x.p - 0.5|p|^2 lies in
      [m[g] - 0.5 max_psq(g) - eps, m[g] - 0.5 min_psq(g) + eps]
    (eps = fp32r error). Since chunks are |p|^2-sorted, the interval width
    0.5(max-min) is tiny. Interval logic gives a small exact candidate set;
    exact float64 rescore of candidate chunks picks the winner; gather rows.
"""

import sys
import types

sys.path.insert(0, "/opt/trn_rl_repo")


def _install_ntff_hook():
    try:
        from trn_agent_boot.trn_boot import _ntff_profile_via_ctypes
    except ImportError:
        return
    try:
        hook = _ntff_profile_via_ctypes("/opt/axon/libaxon_pjrt.so")
    except OSError:
        return
    mod = types.ModuleType("antenv.axon_hooks")
    _h = [hook]
    mod.get_axon_ntff_profile_hook = lambda: _h[0]
    mod.set_axon_ntff_profile_hook = lambda h: _h.__setitem__(0, h)
    sys.modules["antenv.axon_hooks"] = mod
    import antenv

    antenv.axon_hooks = mod


_install_ntff_hook()

import numpy as np
import concourse.bass as bass
import concourse.mybir as mybir
import concourse.tile as tile
from concourse import bacc
from concourse.bass_utils import run_bass_kernel_spmd

B, N, D = 4096, 32768, 256
NCORES = 8
NLOC = N // NCORES  # 4096 prototypes per core
QT = 128  # queries per tile
NQT = B // QT  # 32 query tiles
JC = 512  # j-chunk width (one psum bank)
NJC = NLOC // JC  # 8 banks-worth per core
G = 16  # reduce granularity (chunk width for host rescore)
NG = NLOC // G  # 256 chunk maxes per core

# fp32r absolute-error allowance on m (measured <~1.5e-2 on this data scale)
EPS_FP32R = 0.08


def build(nqt=NQT, njc=NJC):
    """Build the per-core Bass graph. nqt/njc shrinkable for simulation."""
    f32 = mybir.dt.float32
    f32r = mybir.dt.float32r
    nloc = njc * JC
    b = nqt * QT
    hf = max(1, njc // 2)  # banks per psum half
    ng = nloc // G

    nc = bacc.Bacc("TRN2", target_bir_lowering=False, debug=False, num_devices=NCORES)
    xT_d = nc.dram_tensor("xT", [2, 128, b], f32r, kind="ExternalInput").ap()
    pT_d = nc.dram_tensor("pT", [2, 128, nloc], f32r, kind="ExternalInput").ap()
    m_out = nc.dram_tensor("m", [nqt, QT, ng], f32, kind="ExternalOutput").ap()

    with tile.TileContext(nc) as tc:
        with (
            tc.tile_pool(name="persist", bufs=1) as pp,
            tc.tile_pool(name="small", bufs=4) as sp,
            tc.tile_pool(name="ps", bufs=2, space="PSUM") as ps,
        ):
            xT_sb = pp.tile([128, 2, b], f32r)
            pT_sb = pp.tile([128, 2, nloc], f32r)
            # split input DMAs so the first matmuls can start early
            for k in range(2):
                for part in range(4):
                    sl = bass.ts(part, b // 4)
                    nc.sync.dma_start(xT_sb[:, k, sl], xT_d[k][:, sl])
                for part in range(njc):
                    sl = bass.ts(part, JC)
                    nc.sync.dma_start(pT_sb[:, k, sl], pT_d[k][:, sl])

            gph = hf * JC // G  # chunk maxes per half
            for qt in range(nqt):
                qs = bass.ts(qt, QT)
                m_sb = sp.tile([QT, ng], f32, tag="m")
                for h in range(njc // hf):
                    psum_h = ps.tile([QT, hf, JC], f32, tag="psb", name=f"ps{qt}_{h}")
                    for jc in range(hf):
                        for k in range(2):
                            nc.tensor.matmul(
                                psum_h[:, jc, :],
                                xT_sb[:, k, qs],
                                pT_sb[:, k, bass.ts(h * hf + jc, JC)],
                                start=(k == 0),
                                stop=(k == 1),
                            )
                    # per-16-wide-chunk maxes, straight from PSUM
                    nc.vector.tensor_reduce(
                        m_sb[:, h * gph : (h + 1) * gph],
                        psum_h[:].rearrange("q c j -> q (c j)").rearrange(
                            "q (g i) -> q g i", i=G
                        ),
                        axis=mybir.AxisListType.X,
                        op=mybir.AluOpType.max,
                    )
                nc.sync.dma_start(m_out[qt], m_sb[:])
    nc.compile()
    return nc


def _prep_inputs(x, perm_prototypes):
    """Host-side shard prep from the |p|^2-sorted prototype array."""
    xT = np.ascontiguousarray(x.T).reshape(2, 128, B)
    in_maps = []
    for c in range(NCORES):
        P = perm_prototypes[c * NLOC : (c + 1) * NLOC]
        pT = np.ascontiguousarray(P.T).reshape(2, 128, NLOC)
        in_maps.append({"xT": xT, "pT": pT})
    return in_maps


_NC_CACHE = {}


def kernel(x: np.ndarray, prototypes: np.ndarray) -> np.ndarray:
    x = np.asarray(x, dtype=np.float32)
    prototypes = np.asarray(prototypes, dtype=np.float32)
    assert x.shape == (B, D) and prototypes.shape == (N, D)

    if "nc" not in _NC_CACHE:
        _NC_CACHE["nc"] = build()
    nc = _NC_CACHE["nc"]

    # sort prototypes by |p|^2 (host preprocessing / sharding)
    psq = np.einsum("jd,jd->j", prototypes, prototypes)  # fp32
    perm = np.argsort(psq, kind="stable").astype(np.int64)
    P_sorted = prototypes[perm]
    psq_sorted = psq[perm].astype(np.float64)

    in_maps = _prep_inputs(x, P_sorted)
    res = run_bass_kernel_spmd(nc, in_maps, core_ids=list(range(NCORES)))
    _NC_CACHE["last_results"] = res

    # m[c, q, g]: max of x.p over sorted 16-chunk g of core c (fp32r-accurate)
    m_all = np.stack([res.results[c]["m"].reshape(B, NG) for c in range(NCORES)])
    m_flat = np.transpose(m_all, (1, 0, 2)).reshape(B, NCORES * NG).astype(np.float64)

    # interval bounds on each chunk's true max of c' = x.p - 0.5 |p|^2
    psq_ch = psq_sorted.reshape(N // G, G)
    hmin = 0.5 * psq_ch.min(axis=1)  # [2048]
    hmax = 0.5 * psq_ch.max(axis=1)
    ub = m_flat - hmin[None, :] + EPS_FP32R
    lb = m_flat - hmax[None, :] - EPS_FP32R
    best_lb = lb.max(axis=1, keepdims=True)
    qs, gs = np.nonzero(ub >= best_lb)  # exact-coverage candidate chunks

    # exact rescore of candidate chunks in float64 (indices in sorted order)
    cand_sj = (gs[:, None] * G + np.arange(G)[None, :]).reshape(-1)
    qq = np.repeat(qs, G)
    cand_j = perm[cand_sj]  # original prototype indices
    pc = prototypes[cand_j].astype(np.float64)
    xc = x[qq].astype(np.float64)
    c_exact = np.einsum("ij,ij->i", pc, xc) - 0.5 * np.einsum("ij,ij->i", pc, pc)
    order = np.lexsort((cand_j, -c_exact, qq))
    qs_o = qq[order]
    first = np.unique(qs_o, return_index=True)[1]
    out_idx = np.empty(B, dtype=np.int64)
    out_idx[qs_o[first]] = cand_j[order][first]

    return prototypes[out_idx]


if __name__ == "__main__":
    rng = np.random.default_rng(0)
    x = rng.standard_normal((B, D), dtype=np.float32)
    p = rng.standard_normal((N, D), dtype=np.float32)
    out = kernel(x, p)
    print("out", out.shape, out.dtype)
